# revision 29
# baseline (speedup 1.0000x reference)
"""DistanceAttention Trainium2 kernel.

Computes, for x:[B,T,D]:
    v    = x @ W_in.T + b_in
    attn = exp((-|i-j| + padding_mask) / e)        # [B,T,T], no softmax
    out  = attn @ v

Key facts exploited:
  * attn factors as exp(-|i-j|/e) * exp(mask_j/e).  The distance kernel
    r^|i-j| (r = exp(-1/e) ~= 0.692) is < 5e-11 for |i-j| >= 65, far
    below the fp32 resolution of the O(1) outputs, so attn is
    numerically BANDED.  Projecting v on a 64-row-SHIFTED block grid
    (vs_s = v rows [128s-64, 128s+64)) makes each 128-row output block
    a sum of just TWO constant 128x128 matmuls:
        out_m = B_L @ vs_m + B_R @ vs_{m+1},
        B_L[p,u] = r^|p-u+64|,  B_R[p,u] = r^|p-u-64|  (B_R = B_L^T).
  * The whole datapath streams fp16: quantizing x/W/decay/v/out to
    fp16 (products accumulate in fp32 PSUM) gives l2 rel err ~4e-4
    vs the fp32 reference -- 45x inside the 2e-2 gate -- and halves
    every DMA byte (memory-bound regime) while keeping single-pass
    PE matmuls (1 col/cycle, same as f32r).
  * DMA issue slots (~650ns each on the issuing engine) and queue
    dispatch are the DMA bottleneck, not bandwidth (one HWDGE queue
    bursts ~350 GB/s).  Inputs are 4 transfers split across BOTH
    hardware DGE queues (SP + Activation); outputs are 3 merged 2KB-
    per-partition transfers plus a split final pair.
  * Projection runs as one lone block (block 0, in the first transfer)
    plus 8 pairs, so the LAST pair is (15,16) and the kernel tail is
    just [last cast -> last decay pair -> split copies -> split DMAs]
    with no projection left over.
  * PSUM->SBUF evacuation (~1.08ns/col + ~140ns/op) is co-critical
    with the PE: v casts run on the DVE, out-copies alternate
    scalar/DVE, and the terminal pair splits across both.
  * exp(mask/e) is a per-row scale of v and commutes with the
    projection: folded into x on the host.  b_in enters the output as a
    rank-1 term added exactly on the host (zero here; generality path).

Sharding: batch(4) x seq-half(2) -> 8 cores, each owning 2048 rows plus
a 64-row halo per side.  No cross-core communication.
Output is stored p-major ([128, 16*256]) so every out-DMA line is
contiguous; the host transposes back.
"""

import numpy as np

B, T, D = 4, 4096, 256
NCORES = 8
THALF = T // 2  # rows owned per core
HALO = 64
LOC = THALF + 2 * HALO  # local rows incl. halo = 2176
NBLK = LOC // 128  # 17 shifted v blocks
NOUT = THALF // 128  # 16 output blocks
E = float(np.e)

# input stream layout: ALL inputs ride the sync queue serially in
# dependency order -- any concurrent transfer (even on the other DGE
# queue) steals slots from the shared DMA-engine pool and smears the
# completion of the latency-critical h1.  The scalar queue carries
# half the OUTPUT stream instead.
#   h1a (sync): W | x block 0 | decay blocks  (minimal first-matmul deps)
#   h1b (sync): x blocks 1..2
#   xca (sync): x blocks 3..6
#   xcb (sync): x blocks 7..11
#   xcc (sync): x blocks 12..16
H1A_BLOCKS = (0,)
H1B_BLOCKS = (1, 2)
XCA_BLOCKS = (3, 4, 5, 6)
XCB_BLOCKS = (7, 8, 9, 10, 11)
XCC_BLOCKS = (12, 13, 14, 15, 16)

WARM_MM = 12  # fp16 single-pass N=256 warmup matmuls (p-state ramp + DMA lead)
SEM_LO, SEM_HI = 12, 40  # bass kernel sems live here

_CACHE: dict = {}


def _decay_blocks() -> np.ndarray:
    """lhsT-layout decay blocks [128, 2*128]: L | R (fp16).

    matmul(out, lhsT, rhs) computes out[p,n] = sum_q lhsT[q,p] rhs[q,n].
    Out-block m needs  B_L @ vs_m + B_R @ vs_{m+1}  with
      B_L[p,q] = r^|p-q+64|,  B_R[p,q] = r^|p-q-64|
    so lhsT_L[q,p] = B_L[p,q] etc.  Entries are computed exactly like the
    reference (exp(-dist/e) in fp32) then rounded to fp16.
    """
    i = np.arange(128, dtype=np.float64)
    dL = np.abs(i[None, :] - i[:, None] + 64.0)  # lhsT_L[q,p] = B_L[p,q]
    dR = np.abs(i[None, :] - i[:, None] - 64.0)
    dist = np.concatenate([dL, dR], axis=1)
    tg = (-dist.astype(np.float32)) / np.float32(E)
    return np.exp(tg).astype(np.float16)


def _build():
    import concourse.bacc as bacc
    import concourse.mybir as mybir
    from concourse.tile import TileContext

    import concourse.bass as bass_mod

    fp = mybir.dt.float32
    hp = mybir.dt.float16

    # Allocate bass's kernel semaphores LOW (walrus's own live below 12).
    orig_range = bass_mod.get_kernel_semaphore_range
    bass_mod.get_kernel_semaphore_range = lambda: range(SEM_LO, SEM_HI)
    try:
        nc = bacc.Bacc(None, target_bir_lowering=False, debug=False)

        h1w = 2 * D + 2 * len(H1A_BLOCKS) * 128 + 256
        hbw = 2 * len(H1B_BLOCKS) * 128
        xaw = 2 * len(XCA_BLOCKS) * 128
        xbw = 2 * len(XCB_BLOCKS) * 128
        xcw = 2 * len(XCC_BLOCKS) * 128
        h1 = nc.dram_tensor("h1", [128, h1w], hp, kind="ExternalInput")
        h1b = nc.dram_tensor("h1b", [128, hbw], hp, kind="ExternalInput")
        xca = nc.dram_tensor("xca", [128, xaw], hp, kind="ExternalInput")
        xcb = nc.dram_tensor("xcb", [128, xbw], hp, kind="ExternalInput")
        xcc = nc.dram_tensor("xcc", [128, xcw], hp, kind="ExternalInput")
        # p-major output: every DMA line is contiguous; host transposes
        out = nc.dram_tensor("out", [128, NOUT * D], hp, kind="ExternalOutput")

        with TileContext(nc) as tc:
            with (
                tc.tile_pool(name="const", bufs=1) as cpool,
                tc.tile_pool(name="vpool", bufs=1) as vpool,
                tc.tile_pool(name="opool", bufs=1) as opool,
                tc.tile_pool(name="ppsum", bufs=4, space="PSUM") as ppsum,
                tc.tile_pool(name="dpsum", bufs=3, space="PSUM") as dpsum,
            ):
                # PE warmup: dummy matmuls with no data deps run during the
                # DMA lead so the HAM clock gate ramps toward 2.4 GHz by the
                # time the first real matmul issues
                scr_w = cpool.tile([128, 128], hp, tag="scr_w")
                nc.vector.memset(scr_w[:], 0.0)
                scr_x = cpool.tile([128, D], hp, tag="scr_x")
                nc.vector.memset(scr_x[:], 0.0)
                # lone-block PSUM bank: warmup, then projection of block 0
                # (full 2KB bank so later pool tiles stay bank-aligned)
                sps = ppsum.tile([128, 2 * D], fp, tag="sps", bufs=1)
                for _ in range(WARM_MM):
                    nc.tensor.matmul(sps[:, 0:D], scr_w[:], scr_x[:],
                                     start=True, stop=True)

                h1_sb = cpool.tile([128, h1w], hp, tag="h1")
                nc.sync.dma_start(out=h1_sb[:], in_=h1[:])
                hb_sb = cpool.tile([128, hbw], hp, tag="hb")
                nc.sync.dma_start(out=hb_sb[:], in_=h1b[:])
                xa_sb = cpool.tile([128, xaw], hp, tag="xa")
                nc.sync.dma_start(out=xa_sb[:], in_=xca[:])
                xb_sb = cpool.tile([128, xbw], hp, tag="xb")
                nc.sync.dma_start(out=xb_sb[:], in_=xcb[:])
                xc_sb = cpool.tile([128, xcw], hp, tag="xc")
                nc.sync.dma_start(out=xc_sb[:], in_=xcc[:])

                wT_sb = [h1_sb[:, 0:D], h1_sb[:, D:2 * D]]
                mdo = 2 * D + 2 * len(H1A_BLOCKS) * 128
                md_sb = h1_sb[:, mdo:mdo + 256]

                # block -> (buffer, col offset, chunk len, index in chunk)
                bmap = {}
                for i, m in enumerate(H1A_BLOCKS):
                    bmap[m] = (h1_sb, 2 * D, len(H1A_BLOCKS), i)
                for i, m in enumerate(H1B_BLOCKS):
                    bmap[m] = (hb_sb, 0, len(H1B_BLOCKS), i)
                for i, m in enumerate(XCA_BLOCKS):
                    bmap[m] = (xa_sb, 0, len(XCA_BLOCKS), i)
                for i, m in enumerate(XCB_BLOCKS):
                    bmap[m] = (xb_sb, 0, len(XCB_BLOCKS), i)
                for i, m in enumerate(XCC_BLOCKS):
                    bmap[m] = (xc_sb, 0, len(XCC_BLOCKS), i)

                def xap(k, m):  # lhsT for v-block m, d-half k
                    buf, off, nb, i = bmap[m]
                    c = off + (k * nb + i) * 128
                    return buf[:, c:c + 128]

                # all 17 v blocks in one tile so any 512-wide window
                # [vs_s | vs_s+1] is a contiguous rhs
                v_sb = vpool.tile([128, NBLK * D], hp, tag="v")
                o_sb = opool.tile([128, NOUT * D], hp, tag="o")
                pps = [ppsum.tile([128, 2 * D], fp, name=f"pp{i}",
                                  tag=f"pp{i}", bufs=1) for i in range(3)]
                dps = [dpsum.tile([128, 2 * D], fp, name=f"dp{i}",
                                  tag=f"dp{i}", bufs=1) for i in range(3)]

                def proj0():
                    # lone block 0 right out of h1; reuses the warmup bank.
                    # cast on scalar (its act table loads in the prologue)
                    nc.tensor.matmul(sps[:, 0:D], xap(0, 0), wT_sb[0][:],
                                     start=True, stop=False)
                    nc.tensor.matmul(sps[:, 0:D], xap(1, 0), wT_sb[1][:],
                                     start=False, stop=True)
                    nc.scalar.copy(v_sb[:, 0:D], sps[:, 0:D])

                def proj_pair(p):
                    # project blocks (2p-1, 2p) into one [128, 512] PSUM pair
                    a = 2 * p - 1
                    pp = pps[(p - 1) % 3]
                    nc.tensor.matmul(pp[:, 0:D], xap(0, a), wT_sb[0][:],
                                     start=True, stop=False)
                    nc.tensor.matmul(pp[:, D:2 * D], xap(0, a + 1), wT_sb[0][:],
                                     start=False, stop=False)
                    nc.tensor.matmul(pp[:, 0:D], xap(1, a), wT_sb[1][:],
                                     start=False, stop=False)
                    nc.tensor.matmul(pp[:, D:2 * D], xap(1, a + 1), wT_sb[1][:],
                                     start=False, stop=True)
                    if p == 8:
                        # last pair: halve the cast on the critical tail
                        nc.vector.tensor_copy(v_sb[:, a * D:(a + 1) * D],
                                              pp[:, 0:D])
                        nc.scalar.copy(v_sb[:, (a + 1) * D:(a + 2) * D],
                                       pp[:, D:2 * D])
                    else:
                        nc.vector.tensor_copy(v_sb[:, a * D:(a + 2) * D], pp[:])

                def decay_pair(a, copy_eng=None, last=False):
                    # out blocks (a, a+1) as one [128, 512] PSUM pair
                    dp = dps[(a // 2) % 3]
                    nc.tensor.matmul(dp[:], md_sb[:, 0:128],
                                     v_sb[:, a * D:(a + 2) * D],
                                     start=True, stop=False)
                    nc.tensor.matmul(dp[:], md_sb[:, 128:256],
                                     v_sb[:, (a + 1) * D:(a + 3) * D],
                                     start=False, stop=True)
                    ob = o_sb[:, a * D:(a + 2) * D]
                    if last:
                        # split the terminal copy+DMA chain across engines
                        # and both DGE queues
                        nc.vector.tensor_copy(ob[:, 0:D], dp[:, 0:D])
                        nc.scalar.copy(ob[:, D:2 * D], dp[:, D:2 * D])
                        dst = out[:, a * D:(a + 2) * D]
                        nc.sync.dma_start(out=dst[:, 0:D], in_=ob[:, 0:D])
                        nc.scalar.dma_start(out=dst[:, D:2 * D],
                                            in_=ob[:, D:2 * D])
                    else:
                        copy_eng(ob, dp[:])

                def out_dma(eng, a0, a1):
                    # one merged DMA for out blocks [a0, a1)
                    eng.dma_start(out=out[:, a0 * D:a1 * D],
                                  in_=o_sb[:, a0 * D:a1 * D])

                # interleave: decay pair (a, a+1) unlocks after the cast of
                # proj pair (a+2)//2+1 -- emit each decay right behind the
                # proj pair that feeds it; out DMAs merge two decay pairs
                # and alternate between the two DGE queues
                proj0()
                proj_pair(1)
                proj_pair(2)
                decay_pair(0, nc.scalar.copy)
                proj_pair(3)
                decay_pair(2, nc.scalar.copy)
                out_dma(nc.sync, 0, 4)
                proj_pair(4)
                decay_pair(4, nc.scalar.copy)
                proj_pair(5)
                decay_pair(6, nc.scalar.copy)
                out_dma(nc.scalar, 4, 8)
                proj_pair(6)
                decay_pair(8, nc.scalar.copy)
                proj_pair(7)
                decay_pair(10, nc.scalar.copy)
                out_dma(nc.sync, 8, 12)
                proj_pair(8)
                decay_pair(12, nc.vector.tensor_copy)
                out_dma(nc.scalar, 12, 14)
                decay_pair(14, last=True)

        nc.compile()
    finally:
        bass_mod.get_kernel_semaphore_range = orig_range
    return nc


def _shard_inputs(x, padding_mask, W_in, b_in):
    x = np.asarray(x, np.float32)
    padding_mask = np.asarray(padding_mask, np.float32)
    if np.any(padding_mask):
        x = x * np.exp(padding_mask / np.float32(E)).transpose(0, 2, 1)
    x = x.astype(np.float16)
    wT = np.asarray(W_in, np.float32).astype(np.float16).T.reshape(2, 128, D)
    wpack = np.concatenate([wT[0], wT[1]], axis=1)  # [128, 2D]
    mdec = _decay_blocks()
    in_maps = []
    for c in range(NCORES):
        bidx, half = divmod(c, 2)
        start = half * THALF
        lo, hi = start - HALO, start + THALF + HALO
        glo, ghi = max(lo, 0), min(hi, T)
        xsl = np.zeros((LOC, D), np.float16)
        xsl[glo - lo:ghi - lo] = x[bidx, glo:ghi]
        xTc = np.ascontiguousarray(xsl.T).reshape(2, 128, LOC)

        def chunk(blocks):  # [128, 2*128*len]: both d-halves side by side
            cols = np.concatenate(
                [xTc[:, :, 128 * m:128 * (m + 1)] for m in blocks], axis=2)
            return np.concatenate([cols[0], cols[1]], axis=1)

        im = {
            "h1": np.ascontiguousarray(
                np.concatenate([wpack, chunk(H1A_BLOCKS), mdec], axis=1)),
            "h1b": np.ascontiguousarray(chunk(H1B_BLOCKS)),
            "xca": np.ascontiguousarray(chunk(XCA_BLOCKS)),
            "xcb": np.ascontiguousarray(chunk(XCB_BLOCKS)),
            "xcc": np.ascontiguousarray(chunk(XCC_BLOCKS)),
        }
        in_maps.append(im)
    return in_maps


def _bias_correction(out, padding_mask, b_in):
    """out += attn @ (1 (x) b_in) = (attn_dist @ exp(mask/e)) (x) b_in."""
    b_in = np.asarray(b_in, np.float32)
    if not np.any(b_in):
        return
    k = np.arange(-256, 257, dtype=np.float32)
    w = np.exp(-np.abs(k) / np.float32(E)).astype(np.float64)
    s_all = np.exp(np.asarray(padding_mask, np.float32)[:, 0, :]
                   / np.float32(E)).astype(np.float64)
    for bidx in range(B):
        a = np.convolve(s_all[bidx], w, mode="same").astype(np.float32)
        out[bidx] += np.outer(a, b_in)


def _patched_walrus(bu):
    """Scoped --max-sem-num injection for the NEFF build; restored by
    kernel() right after the run."""
    orig = bu.get_walrus_args

    def patched(*args, **kwargs):
        return [f"--max-sem-num={SEM_HI}", *orig(*args, **kwargs)]

    bu.get_walrus_args = patched
    return orig


def kernel(x, padding_mask, W_in, b_in):
    import concourse.bass_utils as bu
    from concourse.bass_utils import run_bass_kernel_spmd

    if "nc" not in _CACHE:
        _CACHE["nc"] = _build()
    nc = _CACHE["nc"]

    in_maps = _shard_inputs(x, padding_mask, W_in, b_in)
    orig_walrus_args = _patched_walrus(bu)
    try:
        res = run_bass_kernel_spmd(nc, in_maps, list(range(NCORES)))
    finally:
        bu.get_walrus_args = orig_walrus_args
    out = np.empty((B, T, D), np.float32)
    for c in range(NCORES):
        bidx, half = divmod(c, 2)
        o = res.results[c]["out"].reshape(128, NOUT, D).astype(np.float32)
        out[bidx, half * THALF:(half + 1) * THALF] = \
            o.transpose(1, 0, 2).reshape(THALF, D)
    _bias_correction(out, padding_mask, b_in)
    return out


# revision 30
# speedup vs baseline: 1.0979x; 1.0979x over previous
"""DistanceAttention Trainium2 kernel.

Computes, for x:[B,T,D]:
    v    = x @ W_in.T + b_in
    attn = exp((-|i-j| + padding_mask) / e)        # [B,T,T], no softmax
    out  = attn @ v

Key facts exploited:
  * attn factors as exp(-|i-j|/e) * exp(mask_j/e).  The distance kernel
    r^|i-j| (r = exp(-1/e) ~= 0.692) is < 5e-11 for |i-j| >= 65, far
    below the fp32 resolution of the O(1) outputs, so attn is
    numerically BANDED.  Projecting v on a 64-row-SHIFTED block grid
    (vs_s = v rows [128s-64, 128s+64)) makes each 128-row output block
    a sum of just TWO constant 128x128 matmuls:
        out_m = B_L @ vs_m + B_R @ vs_{m+1},
        B_L[p,u] = r^|p-u+64|,  B_R[p,u] = r^|p-u-64|  (B_R = B_L^T).
  * The whole datapath streams fp16: quantizing x/W/decay/v/out to
    fp16 (products accumulate in fp32 PSUM) gives l2 rel err ~4e-4
    vs the fp32 reference -- 45x inside the 2e-2 gate -- and halves
    every DMA byte (memory-bound regime) while keeping single-pass
    PE matmuls (1 col/cycle, same as f32r).
  * DMA issue slots (~650ns each on the issuing engine) and queue
    dispatch are the DMA bottleneck, not bandwidth (one HWDGE queue
    bursts ~350 GB/s).  Inputs are 4 transfers split across BOTH
    hardware DGE queues (SP + Activation); outputs are 3 merged 2KB-
    per-partition transfers plus a split final pair.
  * Projection runs as one lone block (block 0, in the first transfer)
    plus 8 pairs, so the LAST pair is (15,16) and the kernel tail is
    just [last cast -> last decay pair -> split copies -> split DMAs]
    with no projection left over.
  * PSUM->SBUF evacuation (~1.08ns/col + ~140ns/op) is co-critical
    with the PE: v casts run on the DVE, out-copies alternate
    scalar/DVE, and the terminal pair splits across both.
  * exp(mask/e) is a per-row scale of v and commutes with the
    projection: folded into x on the host.  b_in enters the output as a
    rank-1 term added exactly on the host (zero here; generality path).

Sharding: batch(4) x seq-half(2) -> 8 cores, each owning 2048 rows plus
a 64-row halo per side.  No cross-core communication.
Output is stored p-major ([128, 16*256]) so every out-DMA line is
contiguous; the host transposes back.
"""

import numpy as np

B, T, D = 4, 4096, 256
NCORES = 8
THALF = T // 2  # rows owned per core
HALO = 64
LOC = THALF + 2 * HALO  # local rows incl. halo = 2176
NBLK = LOC // 128  # 17 shifted v blocks
NOUT = THALF // 128  # 16 output blocks
E = float(np.e)

# input stream layout: ALL inputs ride the sync queue serially in
# dependency order -- any concurrent transfer (even on the other DGE
# queue) steals slots from the shared DMA-engine pool and smears the
# completion of the latency-critical h1.  The scalar queue carries
# half the OUTPUT stream instead.
#   h1  (sync): W | x blocks 0..2 | decay blocks
#   xca (sync): x blocks 3..6
#   xcb (sync): x blocks 7..11
#   xcc (sync): x blocks 12..16
H1_BLOCKS = (0, 1, 2)
XCA_BLOCKS = (3, 4, 5, 6)
XCB_BLOCKS = (7, 8, 9, 10, 11)
XCC_BLOCKS = (12, 13, 14, 15, 16)

WARM_MM = 16  # fp16 single-pass N=256 warmup matmuls (p-state ramp + DMA lead)
SEM_LO, SEM_HI = 12, 40  # bass kernel sems live here

_CACHE: dict = {}


def _decay_blocks() -> np.ndarray:
    """lhsT-layout decay blocks [128, 2*128]: L | R (fp16).

    matmul(out, lhsT, rhs) computes out[p,n] = sum_q lhsT[q,p] rhs[q,n].
    Out-block m needs  B_L @ vs_m + B_R @ vs_{m+1}  with
      B_L[p,q] = r^|p-q+64|,  B_R[p,q] = r^|p-q-64|
    so lhsT_L[q,p] = B_L[p,q] etc.  Entries are computed exactly like the
    reference (exp(-dist/e) in fp32) then rounded to fp16.
    """
    i = np.arange(128, dtype=np.float64)
    dL = np.abs(i[None, :] - i[:, None] + 64.0)  # lhsT_L[q,p] = B_L[p,q]
    dR = np.abs(i[None, :] - i[:, None] - 64.0)
    dist = np.concatenate([dL, dR], axis=1)
    tg = (-dist.astype(np.float32)) / np.float32(E)
    return np.exp(tg).astype(np.float16)


def _build():
    import concourse.bacc as bacc
    import concourse.mybir as mybir
    from concourse.tile import TileContext

    import concourse.bass as bass_mod

    fp = mybir.dt.float32
    hp = mybir.dt.float16

    # Allocate bass's kernel semaphores LOW (walrus's own live below 12).
    orig_range = bass_mod.get_kernel_semaphore_range
    bass_mod.get_kernel_semaphore_range = lambda: range(SEM_LO, SEM_HI)
    try:
        nc = bacc.Bacc(None, target_bir_lowering=False, debug=False)

        h1w = 2 * D + 2 * len(H1_BLOCKS) * 128 + 256
        xaw = 2 * len(XCA_BLOCKS) * 128
        xbw = 2 * len(XCB_BLOCKS) * 128
        xcw = 2 * len(XCC_BLOCKS) * 128
        h1 = nc.dram_tensor("h1", [128, h1w], hp, kind="ExternalInput")
        xca = nc.dram_tensor("xca", [128, xaw], hp, kind="ExternalInput")
        xcb = nc.dram_tensor("xcb", [128, xbw], hp, kind="ExternalInput")
        xcc = nc.dram_tensor("xcc", [128, xcw], hp, kind="ExternalInput")
        # p-major output: every DMA line is contiguous; host transposes
        out = nc.dram_tensor("out", [128, NOUT * D], hp, kind="ExternalOutput")

        with TileContext(nc) as tc:
            with (
                tc.tile_pool(name="const", bufs=1) as cpool,
                tc.tile_pool(name="vpool", bufs=1) as vpool,
                tc.tile_pool(name="opool", bufs=1) as opool,
                tc.tile_pool(name="ppsum", bufs=4, space="PSUM") as ppsum,
                tc.tile_pool(name="dpsum", bufs=3, space="PSUM") as dpsum,
            ):
                # PE warmup: dummy matmuls with no data deps run during the
                # DMA lead so the HAM clock gate ramps toward 2.4 GHz by the
                # time the first real matmul issues
                scr_w = cpool.tile([128, 128], hp, tag="scr_w")
                nc.vector.memset(scr_w[:], 0.0)
                scr_x = cpool.tile([128, D], hp, tag="scr_x")
                nc.vector.memset(scr_x[:], 0.0)
                # lone-block PSUM bank: warmup, then projection of block 0
                # (full 2KB bank so later pool tiles stay bank-aligned)
                sps = ppsum.tile([128, 2 * D], fp, tag="sps", bufs=1)
                for _ in range(WARM_MM):
                    nc.tensor.matmul(sps[:, 0:D], scr_w[:], scr_x[:],
                                     start=True, stop=True)

                h1_sb = cpool.tile([128, h1w], hp, tag="h1")
                nc.sync.dma_start(out=h1_sb[:], in_=h1[:])
                xa_sb = cpool.tile([128, xaw], hp, tag="xa")
                nc.sync.dma_start(out=xa_sb[:], in_=xca[:])
                xb_sb = cpool.tile([128, xbw], hp, tag="xb")
                nc.sync.dma_start(out=xb_sb[:], in_=xcb[:])
                xc_sb = cpool.tile([128, xcw], hp, tag="xc")
                nc.sync.dma_start(out=xc_sb[:], in_=xcc[:])

                wT_sb = [h1_sb[:, 0:D], h1_sb[:, D:2 * D]]
                mdo = 2 * D + 2 * len(H1_BLOCKS) * 128
                md_sb = h1_sb[:, mdo:mdo + 256]

                # block -> (buffer, col offset, chunk len, index in chunk)
                bmap = {}
                for i, m in enumerate(H1_BLOCKS):
                    bmap[m] = (h1_sb, 2 * D, len(H1_BLOCKS), i)
                for i, m in enumerate(XCA_BLOCKS):
                    bmap[m] = (xa_sb, 0, len(XCA_BLOCKS), i)
                for i, m in enumerate(XCB_BLOCKS):
                    bmap[m] = (xb_sb, 0, len(XCB_BLOCKS), i)
                for i, m in enumerate(XCC_BLOCKS):
                    bmap[m] = (xc_sb, 0, len(XCC_BLOCKS), i)

                def xap(k, m):  # lhsT for v-block m, d-half k
                    buf, off, nb, i = bmap[m]
                    c = off + (k * nb + i) * 128
                    return buf[:, c:c + 128]

                # all 17 v blocks in one tile so any 512-wide window
                # [vs_s | vs_s+1] is a contiguous rhs
                v_sb = vpool.tile([128, NBLK * D], hp, tag="v")
                o_sb = opool.tile([128, NOUT * D], hp, tag="o")
                pps = [ppsum.tile([128, 2 * D], fp, name=f"pp{i}",
                                  tag=f"pp{i}", bufs=1) for i in range(3)]
                dps = [dpsum.tile([128, 2 * D], fp, name=f"dp{i}",
                                  tag=f"dp{i}", bufs=1) for i in range(3)]

                def proj0():
                    # lone block 0 right out of h1; reuses the warmup bank.
                    # cast on scalar (its act table loads in the prologue)
                    nc.tensor.matmul(sps[:, 0:D], xap(0, 0), wT_sb[0][:],
                                     start=True, stop=False)
                    nc.tensor.matmul(sps[:, 0:D], xap(1, 0), wT_sb[1][:],
                                     start=False, stop=True)
                    nc.scalar.copy(v_sb[:, 0:D], sps[:, 0:D])

                def proj_pair(p):
                    # project blocks (2p-1, 2p) into one [128, 512] PSUM pair
                    a = 2 * p - 1
                    pp = pps[(p - 1) % 3]
                    nc.tensor.matmul(pp[:, 0:D], xap(0, a), wT_sb[0][:],
                                     start=True, stop=False)
                    nc.tensor.matmul(pp[:, D:2 * D], xap(0, a + 1), wT_sb[0][:],
                                     start=False, stop=False)
                    nc.tensor.matmul(pp[:, 0:D], xap(1, a), wT_sb[1][:],
                                     start=False, stop=False)
                    nc.tensor.matmul(pp[:, D:2 * D], xap(1, a + 1), wT_sb[1][:],
                                     start=False, stop=True)
                    if p == 8:
                        # last pair: halve the cast on the critical tail
                        nc.vector.tensor_copy(v_sb[:, a * D:(a + 1) * D],
                                              pp[:, 0:D])
                        nc.scalar.copy(v_sb[:, (a + 1) * D:(a + 2) * D],
                                       pp[:, D:2 * D])
                    else:
                        nc.vector.tensor_copy(v_sb[:, a * D:(a + 2) * D], pp[:])

                def decay_pair(a, copy_eng=None, last=False):
                    # out blocks (a, a+1) as one [128, 512] PSUM pair
                    dp = dps[(a // 2) % 3]
                    nc.tensor.matmul(dp[:], md_sb[:, 0:128],
                                     v_sb[:, a * D:(a + 2) * D],
                                     start=True, stop=False)
                    nc.tensor.matmul(dp[:], md_sb[:, 128:256],
                                     v_sb[:, (a + 1) * D:(a + 3) * D],
                                     start=False, stop=True)
                    ob = o_sb[:, a * D:(a + 2) * D]
                    if last:
                        # split the terminal copy+DMA chain across engines
                        # and both DGE queues
                        nc.vector.tensor_copy(ob[:, 0:D], dp[:, 0:D])
                        nc.scalar.copy(ob[:, D:2 * D], dp[:, D:2 * D])
                        dst = out[:, a * D:(a + 2) * D]
                        nc.sync.dma_start(out=dst[:, 0:D], in_=ob[:, 0:D])
                        nc.scalar.dma_start(out=dst[:, D:2 * D],
                                            in_=ob[:, D:2 * D])
                    else:
                        copy_eng(ob, dp[:])

                def out_dma(eng, a0, a1):
                    # one merged DMA for out blocks [a0, a1)
                    eng.dma_start(out=out[:, a0 * D:a1 * D],
                                  in_=o_sb[:, a0 * D:a1 * D])

                # interleave: decay pair (a, a+1) unlocks after the cast of
                # proj pair (a+2)//2+1 -- emit each decay right behind the
                # proj pair that feeds it; out DMAs merge two decay pairs
                # and alternate between the two DGE queues
                proj0()
                proj_pair(1)
                proj_pair(2)
                decay_pair(0, nc.scalar.copy)
                proj_pair(3)
                decay_pair(2, nc.scalar.copy)
                out_dma(nc.sync, 0, 4)
                proj_pair(4)
                decay_pair(4, nc.scalar.copy)
                proj_pair(5)
                decay_pair(6, nc.scalar.copy)
                out_dma(nc.scalar, 4, 8)
                proj_pair(6)
                decay_pair(8, nc.scalar.copy)
                proj_pair(7)
                decay_pair(10, nc.scalar.copy)
                out_dma(nc.sync, 8, 12)
                proj_pair(8)
                decay_pair(12, nc.vector.tensor_copy)
                out_dma(nc.scalar, 12, 14)
                decay_pair(14, last=True)

        nc.compile()
    finally:
        bass_mod.get_kernel_semaphore_range = orig_range
    return nc


def _shard_inputs(x, padding_mask, W_in, b_in):
    x = np.asarray(x, np.float32)
    padding_mask = np.asarray(padding_mask, np.float32)
    if np.any(padding_mask):
        x = x * np.exp(padding_mask / np.float32(E)).transpose(0, 2, 1)
    x = x.astype(np.float16)
    wT = np.asarray(W_in, np.float32).astype(np.float16).T.reshape(2, 128, D)
    wpack = np.concatenate([wT[0], wT[1]], axis=1)  # [128, 2D]
    mdec = _decay_blocks()
    in_maps = []
    for c in range(NCORES):
        bidx, half = divmod(c, 2)
        start = half * THALF
        lo, hi = start - HALO, start + THALF + HALO
        glo, ghi = max(lo, 0), min(hi, T)
        xsl = np.zeros((LOC, D), np.float16)
        xsl[glo - lo:ghi - lo] = x[bidx, glo:ghi]
        xTc = np.ascontiguousarray(xsl.T).reshape(2, 128, LOC)

        def chunk(blocks):  # [128, 2*128*len]: both d-halves side by side
            cols = np.concatenate(
                [xTc[:, :, 128 * m:128 * (m + 1)] for m in blocks], axis=2)
            return np.concatenate([cols[0], cols[1]], axis=1)

        im = {
            "h1": np.ascontiguousarray(
                np.concatenate([wpack, chunk(H1_BLOCKS), mdec], axis=1)),
            "xca": np.ascontiguousarray(chunk(XCA_BLOCKS)),
            "xcb": np.ascontiguousarray(chunk(XCB_BLOCKS)),
            "xcc": np.ascontiguousarray(chunk(XCC_BLOCKS)),
        }
        in_maps.append(im)
    return in_maps


def _bias_correction(out, padding_mask, b_in):
    """out += attn @ (1 (x) b_in) = (attn_dist @ exp(mask/e)) (x) b_in."""
    b_in = np.asarray(b_in, np.float32)
    if not np.any(b_in):
        return
    k = np.arange(-256, 257, dtype=np.float32)
    w = np.exp(-np.abs(k) / np.float32(E)).astype(np.float64)
    s_all = np.exp(np.asarray(padding_mask, np.float32)[:, 0, :]
                   / np.float32(E)).astype(np.float64)
    for bidx in range(B):
        a = np.convolve(s_all[bidx], w, mode="same").astype(np.float32)
        out[bidx] += np.outer(a, b_in)


def _patched_walrus(bu):
    """Scoped --max-sem-num injection for the NEFF build; restored by
    kernel() right after the run."""
    orig = bu.get_walrus_args

    def patched(*args, **kwargs):
        return [f"--max-sem-num={SEM_HI}", *orig(*args, **kwargs)]

    bu.get_walrus_args = patched
    return orig


def kernel(x, padding_mask, W_in, b_in):
    import concourse.bass_utils as bu
    from concourse.bass_utils import run_bass_kernel_spmd

    if "nc" not in _CACHE:
        _CACHE["nc"] = _build()
    nc = _CACHE["nc"]

    in_maps = _shard_inputs(x, padding_mask, W_in, b_in)
    orig_walrus_args = _patched_walrus(bu)
    try:
        res = run_bass_kernel_spmd(nc, in_maps, list(range(NCORES)))
    finally:
        bu.get_walrus_args = orig_walrus_args
    out = np.empty((B, T, D), np.float32)
    for c in range(NCORES):
        bidx, half = divmod(c, 2)
        o = res.results[c]["out"].reshape(128, NOUT, D).astype(np.float32)
        out[bidx, half * THALF:(half + 1) * THALF] = \
            o.transpose(1, 0, 2).reshape(THALF, D)
    _bias_correction(out, padding_mask, b_in)
    return out


# revision 31
# speedup vs baseline: 1.1133x; 1.0140x over previous
"""DistanceAttention Trainium2 kernel.

Computes, for x:[B,T,D]:
    v    = x @ W_in.T + b_in
    attn = exp((-|i-j| + padding_mask) / e)        # [B,T,T], no softmax
    out  = attn @ v

Key facts exploited:
  * attn factors as exp(-|i-j|/e) * exp(mask_j/e).  The distance kernel
    r^|i-j| (r = exp(-1/e) ~= 0.692) is < 5e-11 for |i-j| >= 65, far
    below the fp32 resolution of the O(1) outputs, so attn is
    numerically BANDED.  Projecting v on a 64-row-SHIFTED block grid
    (vs_s = v rows [128s-64, 128s+64)) makes each 128-row output block
    a sum of just TWO constant 128x128 matmuls:
        out_m = B_L @ vs_m + B_R @ vs_{m+1},
        B_L[p,u] = r^|p-u+64|,  B_R[p,u] = r^|p-u-64|  (B_R = B_L^T).
  * The whole datapath streams fp16: quantizing x/W/decay/v/out to
    fp16 (products accumulate in fp32 PSUM) gives l2 rel err ~4e-4
    vs the fp32 reference -- 45x inside the 2e-2 gate -- and halves
    every DMA byte (memory-bound regime) while keeping single-pass
    PE matmuls (1 col/cycle, same as f32r).
  * DMA issue slots (~650ns each on the issuing engine) and queue
    dispatch are the DMA bottleneck, not bandwidth (one HWDGE queue
    bursts ~350 GB/s).  Inputs are 4 transfers split across BOTH
    hardware DGE queues (SP + Activation); outputs are 3 merged 2KB-
    per-partition transfers plus a split final pair.
  * Projection runs as one lone block (block 0, in the first transfer)
    plus 8 pairs, so the LAST pair is (15,16) and the kernel tail is
    just [last cast -> last decay pair -> split copies -> split DMAs]
    with no projection left over.
  * PSUM->SBUF evacuation (~1.08ns/col + ~140ns/op) is co-critical
    with the PE: v casts run on the DVE, out-copies alternate
    scalar/DVE, and the terminal pair splits across both.
  * exp(mask/e) is a per-row scale of v and commutes with the
    projection: folded into x on the host.  b_in enters the output as a
    rank-1 term added exactly on the host (zero here; generality path).

Sharding: batch(4) x seq-half(2) -> 8 cores, each owning 2048 rows plus
a 64-row halo per side.  No cross-core communication.
Output is stored p-major ([128, 16*256]) so every out-DMA line is
contiguous; the host transposes back.
"""

import numpy as np

B, T, D = 4, 4096, 256
NCORES = 8
THALF = T // 2  # rows owned per core
HALO = 64
LOC = THALF + 2 * HALO  # local rows incl. halo = 2176
NBLK = LOC // 128  # 17 shifted v blocks
NOUT = THALF // 128  # 16 output blocks
E = float(np.e)

# input stream layout: ALL inputs ride the sync queue serially in
# dependency order -- any concurrent transfer (even on the other DGE
# queue) steals slots from the shared DMA-engine pool and smears the
# completion of the latency-critical h1.  The scalar queue carries
# half the OUTPUT stream instead.
#   h1  (sync): W | x blocks 0..2 | decay blocks
#   xca (sync): x blocks 3..6
#   xcb (sync): x blocks 7..11
#   xcc (sync): x blocks 12..16
H1_BLOCKS = (0, 1, 2)
XCA_BLOCKS = (3, 4, 5, 6)
XCB_BLOCKS = (7, 8, 9, 10, 11)
XCC_BLOCKS = (12, 13, 14, 15, 16)

WARM_MM = 16  # fp16 single-pass N=256 warmup matmuls (p-state ramp + DMA lead)
SEM_LO, SEM_HI = 12, 40  # bass kernel sems live here

_CACHE: dict = {}


def _decay_blocks() -> np.ndarray:
    """lhsT-layout decay blocks [128, 2*128]: L | R (fp16).

    matmul(out, lhsT, rhs) computes out[p,n] = sum_q lhsT[q,p] rhs[q,n].
    Out-block m needs  B_L @ vs_m + B_R @ vs_{m+1}  with
      B_L[p,q] = r^|p-q+64|,  B_R[p,q] = r^|p-q-64|
    so lhsT_L[q,p] = B_L[p,q] etc.  Entries are computed exactly like the
    reference (exp(-dist/e) in fp32) then rounded to fp16.
    """
    i = np.arange(128, dtype=np.float64)
    dL = np.abs(i[None, :] - i[:, None] + 64.0)  # lhsT_L[q,p] = B_L[p,q]
    dR = np.abs(i[None, :] - i[:, None] - 64.0)
    dist = np.concatenate([dL, dR], axis=1)
    tg = (-dist.astype(np.float32)) / np.float32(E)
    return np.exp(tg).astype(np.float16)


def _build():
    import concourse.bacc as bacc
    import concourse.mybir as mybir
    from concourse.tile import TileContext

    import concourse.bass as bass_mod

    fp = mybir.dt.float32
    hp = mybir.dt.float16

    # Allocate bass's kernel semaphores LOW (walrus's own live below 12).
    orig_range = bass_mod.get_kernel_semaphore_range
    bass_mod.get_kernel_semaphore_range = lambda: range(SEM_LO, SEM_HI)
    try:
        nc = bacc.Bacc(None, target_bir_lowering=False, debug=False)

        h1w = 2 * D + 2 * len(H1_BLOCKS) * 128 + 256
        xaw = 2 * len(XCA_BLOCKS) * 128
        xbw = 2 * len(XCB_BLOCKS) * 128
        xcw = 2 * len(XCC_BLOCKS) * 128
        h1 = nc.dram_tensor("h1", [128, h1w], hp, kind="ExternalInput")
        xca = nc.dram_tensor("xca", [128, xaw], hp, kind="ExternalInput")
        xcb = nc.dram_tensor("xcb", [128, xbw], hp, kind="ExternalInput")
        xcc = nc.dram_tensor("xcc", [128, xcw], hp, kind="ExternalInput")
        # p-major output: every DMA line is contiguous; host transposes
        out = nc.dram_tensor("out", [128, NOUT * D], hp, kind="ExternalOutput")

        with TileContext(nc) as tc:
            with (
                tc.tile_pool(name="const", bufs=1) as cpool,
                tc.tile_pool(name="vpool", bufs=1) as vpool,
                tc.tile_pool(name="opool", bufs=1) as opool,
                tc.tile_pool(name="ppsum", bufs=4, space="PSUM") as ppsum,
                tc.tile_pool(name="dpsum", bufs=3, space="PSUM") as dpsum,
            ):
                # PE warmup: dummy matmuls with no data deps run during the
                # DMA lead so the HAM clock gate ramps toward 2.4 GHz by the
                # time the first real matmul issues
                scr_w = cpool.tile([128, 128], hp, tag="scr_w")
                nc.vector.memset(scr_w[:], 0.0)
                scr_x = cpool.tile([128, D], hp, tag="scr_x")
                nc.vector.memset(scr_x[:], 0.0)
                # lone-block PSUM bank: warmup, then projection of block 0
                # (full 2KB bank so later pool tiles stay bank-aligned)
                sps = ppsum.tile([128, 2 * D], fp, tag="sps", bufs=1)
                for _ in range(WARM_MM):
                    nc.tensor.matmul(sps[:, 0:D], scr_w[:], scr_x[:],
                                     start=True, stop=True)

                h1_sb = cpool.tile([128, h1w], hp, tag="h1")
                nc.sync.dma_start(out=h1_sb[:], in_=h1[:])
                xa_sb = cpool.tile([128, xaw], hp, tag="xa")
                nc.sync.dma_start(out=xa_sb[:], in_=xca[:])
                xb_sb = cpool.tile([128, xbw], hp, tag="xb")
                nc.sync.dma_start(out=xb_sb[:], in_=xcb[:])
                xc_sb = cpool.tile([128, xcw], hp, tag="xc")
                nc.sync.dma_start(out=xc_sb[:], in_=xcc[:])

                wT_sb = [h1_sb[:, 0:D], h1_sb[:, D:2 * D]]
                mdo = 2 * D + 2 * len(H1_BLOCKS) * 128
                md_sb = h1_sb[:, mdo:mdo + 256]

                # block -> (buffer, col offset, chunk len, index in chunk)
                bmap = {}
                for i, m in enumerate(H1_BLOCKS):
                    bmap[m] = (h1_sb, 2 * D, len(H1_BLOCKS), i)
                for i, m in enumerate(XCA_BLOCKS):
                    bmap[m] = (xa_sb, 0, len(XCA_BLOCKS), i)
                for i, m in enumerate(XCB_BLOCKS):
                    bmap[m] = (xb_sb, 0, len(XCB_BLOCKS), i)
                for i, m in enumerate(XCC_BLOCKS):
                    bmap[m] = (xc_sb, 0, len(XCC_BLOCKS), i)

                def xap(k, m):  # lhsT for v-block m, d-half k
                    buf, off, nb, i = bmap[m]
                    c = off + (k * nb + i) * 128
                    return buf[:, c:c + 128]

                # all 17 v blocks in one tile so any 512-wide window
                # [vs_s | vs_s+1] is a contiguous rhs
                v_sb = vpool.tile([128, NBLK * D], hp, tag="v")
                o_sb = opool.tile([128, NOUT * D], hp, tag="o")
                pps = [ppsum.tile([128, 2 * D], fp, name=f"pp{i}",
                                  tag=f"pp{i}", bufs=1) for i in range(3)]
                dps = [dpsum.tile([128, 2 * D], fp, name=f"dp{i}",
                                  tag=f"dp{i}", bufs=1) for i in range(3)]

                def proj0():
                    # lone block 0 right out of h1; reuses the warmup bank.
                    # cast on scalar (its act table loads in the prologue)
                    nc.tensor.matmul(sps[:, 0:D], xap(0, 0), wT_sb[0][:],
                                     start=True, stop=False)
                    nc.tensor.matmul(sps[:, 0:D], xap(1, 0), wT_sb[1][:],
                                     start=False, stop=True)
                    nc.scalar.copy(v_sb[:, 0:D], sps[:, 0:D])

                def proj_pair(p):
                    # project blocks (2p-1, 2p) into one [128, 512] PSUM pair
                    a = 2 * p - 1
                    pp = pps[(p - 1) % 3]
                    nc.tensor.matmul(pp[:, 0:D], xap(0, a), wT_sb[0][:],
                                     start=True, stop=False)
                    nc.tensor.matmul(pp[:, D:2 * D], xap(0, a + 1), wT_sb[0][:],
                                     start=False, stop=False)
                    nc.tensor.matmul(pp[:, 0:D], xap(1, a), wT_sb[1][:],
                                     start=False, stop=False)
                    nc.tensor.matmul(pp[:, D:2 * D], xap(1, a + 1), wT_sb[1][:],
                                     start=False, stop=True)
                    if p == 8:
                        # last pair: halve the cast on the critical tail
                        nc.vector.tensor_copy(v_sb[:, a * D:(a + 1) * D],
                                              pp[:, 0:D])
                        nc.scalar.copy(v_sb[:, (a + 1) * D:(a + 2) * D],
                                       pp[:, D:2 * D])
                    else:
                        nc.vector.tensor_copy(v_sb[:, a * D:(a + 2) * D], pp[:])

                def decay_pair(a, copy_eng=None, last=False):
                    # out blocks (a, a+1) as one [128, 512] PSUM pair
                    dp = dps[(a // 2) % 3]
                    nc.tensor.matmul(dp[:], md_sb[:, 0:128],
                                     v_sb[:, a * D:(a + 2) * D],
                                     start=True, stop=False)
                    nc.tensor.matmul(dp[:], md_sb[:, 128:256],
                                     v_sb[:, (a + 1) * D:(a + 3) * D],
                                     start=False, stop=True)
                    ob = o_sb[:, a * D:(a + 2) * D]
                    if last:
                        # split the terminal copy+DMA chain across engines
                        # and both DGE queues
                        nc.vector.tensor_copy(ob[:, 0:D], dp[:, 0:D])
                        nc.scalar.copy(ob[:, D:2 * D], dp[:, D:2 * D])
                        dst = out[:, a * D:(a + 2) * D]
                        nc.sync.dma_start(out=dst[:, 0:D], in_=ob[:, 0:D])
                        nc.scalar.dma_start(out=dst[:, D:2 * D],
                                            in_=ob[:, D:2 * D])
                    else:
                        copy_eng(ob, dp[:])

                def out_dma(eng, a0, a1):
                    # one merged DMA for out blocks [a0, a1)
                    eng.dma_start(out=out[:, a0 * D:a1 * D],
                                  in_=o_sb[:, a0 * D:a1 * D])

                # interleave: decay pair (a, a+1) unlocks after the cast of
                # proj pair (a+2)//2+1 -- emit each decay right behind the
                # proj pair that feeds it; out DMAs merge two decay pairs
                # and alternate between the two DGE queues
                proj0()
                proj_pair(1)
                proj_pair(2)
                decay_pair(0, nc.scalar.copy)
                proj_pair(3)
                decay_pair(2, nc.scalar.copy)
                out_dma(nc.sync, 0, 4)
                proj_pair(4)
                decay_pair(4, nc.scalar.copy)
                proj_pair(5)
                decay_pair(6, nc.scalar.copy)
                out_dma(nc.scalar, 4, 8)
                proj_pair(6)
                decay_pair(8, nc.scalar.copy)
                proj_pair(7)
                # emit the last proj pair (and its split casts) BEFORE the
                # remaining decays: D10+D12 matmuls then fill the PE while
                # the split casts drain, so the terminal pair starts gapless
                proj_pair(8)
                decay_pair(10, nc.scalar.copy)
                out_dma(nc.sync, 8, 12)
                decay_pair(12, nc.vector.tensor_copy)
                out_dma(nc.scalar, 12, 14)
                decay_pair(14, last=True)

        nc.compile()
    finally:
        bass_mod.get_kernel_semaphore_range = orig_range
    return nc


def _shard_inputs(x, padding_mask, W_in, b_in):
    x = np.asarray(x, np.float32)
    padding_mask = np.asarray(padding_mask, np.float32)
    if np.any(padding_mask):
        x = x * np.exp(padding_mask / np.float32(E)).transpose(0, 2, 1)
    x = x.astype(np.float16)
    wT = np.asarray(W_in, np.float32).astype(np.float16).T.reshape(2, 128, D)
    wpack = np.concatenate([wT[0], wT[1]], axis=1)  # [128, 2D]
    mdec = _decay_blocks()
    in_maps = []
    for c in range(NCORES):
        bidx, half = divmod(c, 2)
        start = half * THALF
        lo, hi = start - HALO, start + THALF + HALO
        glo, ghi = max(lo, 0), min(hi, T)
        xsl = np.zeros((LOC, D), np.float16)
        xsl[glo - lo:ghi - lo] = x[bidx, glo:ghi]
        xTc = np.ascontiguousarray(xsl.T).reshape(2, 128, LOC)

        def chunk(blocks):  # [128, 2*128*len]: both d-halves side by side
            cols = np.concatenate(
                [xTc[:, :, 128 * m:128 * (m + 1)] for m in blocks], axis=2)
            return np.concatenate([cols[0], cols[1]], axis=1)

        im = {
            "h1": np.ascontiguousarray(
                np.concatenate([wpack, chunk(H1_BLOCKS), mdec], axis=1)),
            "xca": np.ascontiguousarray(chunk(XCA_BLOCKS)),
            "xcb": np.ascontiguousarray(chunk(XCB_BLOCKS)),
            "xcc": np.ascontiguousarray(chunk(XCC_BLOCKS)),
        }
        in_maps.append(im)
    return in_maps


def _bias_correction(out, padding_mask, b_in):
    """out += attn @ (1 (x) b_in) = (attn_dist @ exp(mask/e)) (x) b_in."""
    b_in = np.asarray(b_in, np.float32)
    if not np.any(b_in):
        return
    k = np.arange(-256, 257, dtype=np.float32)
    w = np.exp(-np.abs(k) / np.float32(E)).astype(np.float64)
    s_all = np.exp(np.asarray(padding_mask, np.float32)[:, 0, :]
                   / np.float32(E)).astype(np.float64)
    for bidx in range(B):
        a = np.convolve(s_all[bidx], w, mode="same").astype(np.float32)
        out[bidx] += np.outer(a, b_in)


def _patched_walrus(bu):
    """Scoped --max-sem-num injection for the NEFF build; restored by
    kernel() right after the run."""
    orig = bu.get_walrus_args

    def patched(*args, **kwargs):
        return [f"--max-sem-num={SEM_HI}", *orig(*args, **kwargs)]

    bu.get_walrus_args = patched
    return orig


def kernel(x, padding_mask, W_in, b_in):
    import concourse.bass_utils as bu
    from concourse.bass_utils import run_bass_kernel_spmd

    if "nc" not in _CACHE:
        _CACHE["nc"] = _build()
    nc = _CACHE["nc"]

    in_maps = _shard_inputs(x, padding_mask, W_in, b_in)
    orig_walrus_args = _patched_walrus(bu)
    try:
        res = run_bass_kernel_spmd(nc, in_maps, list(range(NCORES)))
    finally:
        bu.get_walrus_args = orig_walrus_args
    out = np.empty((B, T, D), np.float32)
    for c in range(NCORES):
        bidx, half = divmod(c, 2)
        o = res.results[c]["out"].reshape(128, NOUT, D).astype(np.float32)
        out[bidx, half * THALF:(half + 1) * THALF] = \
            o.transpose(1, 0, 2).reshape(THALF, D)
    _bias_correction(out, padding_mask, b_in)
    return out
